# revision 1
# baseline (speedup 1.0000x reference)
"""LSTM (B=4096, T=512, I=8, H=64) + FC head, data-parallel over 8 NeuronCores.

Strategy (per sharding hint): shard x along batch across the 8 cores,
replicate the tiny LSTM/FC weights, no cross-core communication.

The per-core compute is expressed in JAX and compiled through PJRT/neuronx-cc
for the NeuronCores. The LSTM input projection for all timesteps is done as
one large matmul up front; the sequential recurrence runs as a lax.scan over
T with a [B_local, H] @ [H, 4H] matmul per step (batch stays on the leading
axis so the tensor engine sees well-shaped matmuls).
"""

import numpy as np
import jax
import jax.numpy as jnp
from jax.sharding import Mesh, PartitionSpec as P

try:
    from jax import shard_map as _shard_map  # jax >= 0.6 stable API
except ImportError:
    from jax.experimental.shard_map import shard_map as _shard_map

B, T, I, H = 4096, 512, 8, 64
N_CORES = 8

_mesh = None
_compiled = None


def _get_mesh():
    global _mesh
    if _mesh is None:
        devs = jax.devices()[:N_CORES]
        assert len(devs) == N_CORES, f"need {N_CORES} devices, got {len(devs)}"
        _mesh = Mesh(np.asarray(devs), ("core",))
    return _mesh


def _lstm_local(x, W_ih, W_hh, b_ih, b_hh, W_fc, b_fc):
    # x: [B_local, T, I] on one core
    Bl = x.shape[0]
    # Input projection for all timesteps at once: [T, B_local, 4H]
    xg = jnp.einsum("bti,gi->tbg", x, W_ih) + (b_ih + b_hh)
    W_hh_T = W_hh.T  # [H, 4H]

    def step(carry, xg_t):
        h, c = carry
        gates = xg_t + h @ W_hh_T  # [B_local, 4H]
        i_g = jax.nn.sigmoid(gates[:, 0 * H:1 * H])
        f_g = jax.nn.sigmoid(gates[:, 1 * H:2 * H])
        g_g = jnp.tanh(gates[:, 2 * H:3 * H])
        o_g = jax.nn.sigmoid(gates[:, 3 * H:4 * H])
        c = f_g * c + i_g * g_g
        h = o_g * jnp.tanh(c)
        return (h, c), None

    h0 = jnp.zeros((Bl, H), x.dtype)
    c0 = jnp.zeros((Bl, H), x.dtype)
    (h_T, _), _ = jax.lax.scan(step, (h0, c0), xg)
    return h_T @ W_fc.T + b_fc  # [B_local, 1]


def _build():
    global _compiled
    if _compiled is None:
        mesh = _get_mesh()
        fn = jax.jit(
            _shard_map(
                _lstm_local,
                mesh=mesh,
                in_specs=(P("core"), P(), P(), P(), P(), P(), P()),
                out_specs=P("core"),
            )
        )
        _compiled = fn
    return _compiled


def _kernel_cpu(args):
    cpu = jax.devices("cpu")[0]
    with jax.default_device(cpu):
        out = jax.jit(_lstm_local, backend="cpu")(*args)
        return np.asarray(jax.device_get(out)).astype(np.float32)


def kernel(x, W_ih, W_hh, b_ih, b_hh, W_fc, b_fc):
    args = (
        np.asarray(x, np.float32),
        np.asarray(W_ih, np.float32),
        np.asarray(W_hh, np.float32),
        np.asarray(b_ih, np.float32),
        np.asarray(b_hh, np.float32),
        np.asarray(W_fc, np.float32),
        np.asarray(b_fc, np.float32),
    )
    try:
        fn = _build()
        out = fn(*args)
        return np.asarray(jax.device_get(out)).astype(np.float32)
    except Exception:
        # device-side failure: fall back to CPU execution (correct, slower)
        return _kernel_cpu(args)



# revision 28
# speedup vs baseline: 8.3929x; 8.3929x over previous
"""LSTM (B=4096, T=512, I=8, H=64) + FC head on 8 Trainium2 NeuronCores.

Data-parallel: each core owns 512 batch rows; LSTM/FC weights replicated.
The per-core recurrence is a hand-written Bass/Tile kernel:

  - Gate pre-activations for a step are two PSUM tiles P0=[f;i], P1=[o;g]
    ([128, B] each, gate-stacked on partitions), produced by row-tiled
    matmuls: lhsT [h~-weights; bias] x rhs [h~; ones] at tile (0,0), plus
    lhsT W_ih x rhs x_t^T at tile (64,0), accumulating into one bank.
  - All four gate nonlinearities are ONE tanh ACT instruction over
    [128, 2B] (both PSUM banks); sigmoid gates use s(x)=(1+tanh(x/2))/2
    with the 1/2 pre-folded into their weight columns.
  - The sigmoid affine fixup is fused into DVE scalar_tensor_tensor ops:
      u~ = (ti2+1)*g'   v~ = (tf2+1)*c
  - The cross-partition add c' = 0.5*(u~+v~) runs on the TensorEngine via a
    constant [128,64] summing matrix (0.5 on the two diagonals).
  - h~ = 2h = (to2+1)*tanh(c'); the factor 2 is folded into W_hh (and
    removed in the host-side FC).
  - x arrives bf16 [BL, T*I]; DMA-xbar transposes stage it to SBUF as
    [t*8+i, batch] chunks; one DVE copy per step moves x_t^T [8, BL] into
    the rhs tile (double-buffered).

Everything recurrent is bf16 in SBUF with fp32 PSUM accumulation.
"""

import numpy as np
import ml_dtypes

B, T, I, H = 4096, 512, 8, 64
N_CORES = 8
BL = B // N_CORES          # 512 batch rows per core
C = 2                      # batch chunks per core (pipelining)
BC = BL // C               # 256 batch rows per chunk
TI = T * I                 # 4096 columns of x per batch row
NXT = TI // 128            # 32 transposed x chunks of [128, BL]

_cache = {"nc": None, "run": None}


def _build_nc():
    import concourse.bass as bass
    import concourse.bacc as bacc
    import concourse.tile as tile
    from concourse import mybir

    f32 = mybir.dt.float32
    bf16 = mybir.dt.bfloat16
    Tanh = mybir.ActivationFunctionType.Tanh
    add_op = mybir.AluOpType.add
    mult_op = mybir.AluOpType.mult

    nc = bacc.Bacc(None, target_bir_lowering=False)

    x_d = nc.dram_tensor("x", [BL, TI], f32, kind="ExternalInput")
    w0h_d = nc.dram_tensor("w0h", [65, 128], bf16, kind="ExternalInput")
    w1h_d = nc.dram_tensor("w1h", [65, 128], bf16, kind="ExternalInput")
    # wxk[k] / wxk[8+k]: block-diagonal W_ih selecting sub-step k of an
    # 8-step x group (rows 8k:8k+8 hold W_ih cols for P0 / P1)
    wxk_d = nc.dram_tensor("wxk", [16, 64, 128], bf16, kind="ExternalInput")
    ident_d = nc.dram_tensor("ident", [128, 128], f32, kind="ExternalInput")
    aadd_d = nc.dram_tensor("aadd", [128, 64], bf16, kind="ExternalInput")
    ht_d = nc.dram_tensor("hT", [64, BL], f32, kind="ExternalOutput")

    with tile.TileContext(nc) as tc:
        with (
            tc.tile_pool(name="consts", bufs=1) as consts,
            tc.tile_pool(name="xb", bufs=1) as xbp,
            tc.tile_pool(name="pt", bufs=2, space="PSUM") as ptp,
            tc.tile_pool(name="state", bufs=1) as statep,
            tc.tile_pool(name="work", bufs=2) as workp,
            tc.tile_pool(name="pg", bufs=1, space="PSUM") as pgp,
            tc.tile_pool(name="cp", bufs=1, space="PSUM") as cpp,
        ):
            # ---- constants ----
            w0h = consts.tile([65, 128], bf16, tag="w0h", name="w0h")
            w1h = consts.tile([65, 128], bf16, tag="w1h", name="w1h")
            aadds = consts.tile([128, 64], bf16, tag="aadd", name="aadds")
            idents = consts.tile([128, 128], f32, tag="ident", name="idents")
            nc.scalar.dma_start(out=w0h[:], in_=w0h_d[:])
            nc.scalar.dma_start(out=w1h[:], in_=w1h_d[:])
            nc.scalar.dma_start(out=aadds[:], in_=aadd_d[:])
            nc.scalar.dma_start(out=idents[:], in_=ident_d[:])
            wx0, wx1 = [], []
            for k in range(8):
                a = consts.tile([128, 128], bf16, tag=f"wx0_{k}", name=f"wx0_{k}")
                b = consts.tile([128, 128], bf16, tag=f"wx1_{k}", name=f"wx1_{k}")
                nc.scalar.dma_start(out=a[64:128, :], in_=wxk_d[k])
                nc.scalar.dma_start(out=b[64:128, :], in_=wxk_d[8 + k])
                wx0.append(a)
                wx1.append(b)

            # ---- phase 1: load x fp32 (batch-major) ----
            # xbs[j]: [128, TI] fp32, batch rows 128j..128j+127
            xbs = [xbp.tile([128, TI], f32, tag=f"xb{j}", name=f"xb{j}")
                   for j in range(4)]
            for j in range(4):
                # head DMA small so group 0/1 staging can start early
                nc.sync.dma_start(out=xbs[j][:, 0:128],
                                  in_=x_d[j * 128:(j + 1) * 128, 0:128])
                nc.sync.dma_start(out=xbs[j][:, 128:2048],
                                  in_=x_d[j * 128:(j + 1) * 128, 128:2048])
                nc.sync.dma_start(out=xbs[j][:, 2048:TI],
                                  in_=x_d[j * 128:(j + 1) * 128, 2048:TI])

            # ---- phase 1b: state tiles ----
            # hxm[p]: rows 0:64 h~, row 64 ones
            # xgrp[p]: rows 64:128 = x for 8 steps (row 64+8k+i = x[:, 8j+k, i])
            hxm = [statep.tile([65, BL], bf16, tag=f"hxm{p}", name=f"hxm{p}")
                   for p in range(2)]
            xgrp = [statep.tile([128, BL], bf16, tag=f"xgrp{p}", name=f"xgrp{p}")
                    for p in range(2)]
            for p in range(2):
                nc.vector.memset(hxm[p][0:64, :], 0.0)
                nc.vector.memset(hxm[p][64:65, :], 1.0)
            # on-demand transpose of one 8-step x group into PSUM rows
            # 64:127 (TensorE col-tiled), then one DVE copy into xgrp
            def stage_group(j):
                # out = x_slice.T via regular matmul against identity,
                # col-tiled to land at psum partitions 64:128
                pt = ptp.tile([128, BL], f32, tag="pt", name="pt")
                for jj in range(4):
                    nc.tensor.matmul(
                        pt[64:128, jj * 128:(jj + 1) * 128],
                        xbs[jj][:, 64 * j:64 * j + 64], idents[:],
                        start=True, stop=True, tile_position=(0, 64))
                nc.vector.tensor_copy(xgrp[j % 2][64:128, :], pt[64:128, :])

            # stage x groups 0 and 1 (steps 0..7, 8..15)
            stage_group(0)
            stage_group(1)
            # tiny PE dummies: advance PE's observed DMA-queue clock past
            # every x load so later stage matmuls carry <=2 waits
            ptd = ptp.tile([1, 16], f32, tag="ptd", name="ptd", bufs=1)
            d = 0
            for j in range(4):
                for col in (127, 2047, TI - 1):
                    nc.tensor.matmul(ptd[0:1, d:d + 1], xbs[j][0:1, col:col + 1],
                                     xbs[j][0:1, col:col + 1],
                                     start=True, stop=True)
                    d += 1

            # cp psum: c state, per parity, chunks side by side
            cps = [cpp.tile([64, BL], f32, tag=f"cp{p}", name=f"cp{p}")
                   for p in range(2)]
            nc.vector.memset(cps[0][0:64, :], 0.0)

            # ---- phase 2: recurrence ----
            for t in range(T):
                par, nxt = t % 2, (t + 1) % 2
                pg = pgp.tile([128, 2 * BL], f32, tag="pg", name="pg")
                t12 = workp.tile([128, 2 * BL], bf16, tag="t12", name="t12")
                for ch in range(C):
                    o0 = 2 * ch * BC           # P0 = [f; i] for this chunk
                    o1 = o0 + BC               # P1 = [o; g]
                    cs = slice(ch * BC, (ch + 1) * BC)
                    rhs_h = hxm[par][0:65, cs]
                    rhs_x = xgrp[(t // 8) % 2][64:128, cs]
                    k = t % 8
                    nc.tensor.matmul(pg[:, o0:o0 + BC], w0h[:], rhs_h,
                                     start=True, stop=False, tile_position=(0, 0))
                    nc.tensor.matmul(pg[:, o0:o0 + BC], wx0[k][64:128, :], rhs_x,
                                     start=False, stop=True, tile_position=(64, 0))
                    nc.tensor.matmul(pg[:, o1:o1 + BC], w1h[:], rhs_h,
                                     start=True, stop=False, tile_position=(0, 0))
                    nc.tensor.matmul(pg[:, o1:o1 + BC], wx1[k][64:128, :], rhs_x,
                                     start=False, stop=True, tile_position=(64, 0))
                # all four gates (both chunks) in one tanh
                nc.scalar.activation(t12[:], pg[:], Tanh)

                # stage the x group 4 steps ahead (4 PE transposes + 1 DVE
                # copy per 8 steps)
                if t % 8 == 4 and t + 4 < T:
                    stage_group(t // 8 + 1)

                for ch in range(C):
                    o0 = 2 * ch * BC
                    o1 = o0 + BC
                    cs = slice(ch * BC, (ch + 1) * BC)
                    u = workp.tile([128, BC], bf16, tag=f"u{ch}", name=f"u{ch}")
                    # u~ = (ti2 + 1) * g'   rows 64:128
                    nc.vector.scalar_tensor_tensor(
                        u[64:128, :], t12[64:128, o0:o0 + BC], 1.0,
                        t12[64:128, o1:o1 + BC], op0=add_op, op1=mult_op)
                    # v~ = (tf2 + 1) * c    rows 0:64
                    nc.vector.scalar_tensor_tensor(
                        u[0:64, :], t12[0:64, o0:o0 + BC], 1.0,
                        cps[par][0:64, cs], op0=add_op, op1=mult_op)
                    # c' = 0.5*(u~ + v~)  (cross-partition add on PE)
                    nc.tensor.matmul(cps[nxt][0:64, cs], aadds[:], u[:],
                                     start=True, stop=True)
                    # tc = tanh(c')
                    tct = workp.tile([64, BC], bf16, tag=f"tc{ch}", name=f"tc{ch}")
                    nc.scalar.activation(tct[0:64, :], cps[nxt][0:64, cs], Tanh)
                    # h~ = (to2 + 1) * tc
                    nc.vector.scalar_tensor_tensor(
                        hxm[nxt][0:64, cs], t12[0:64, o1:o1 + BC], 1.0,
                        tct[0:64, :], op0=add_op, op1=mult_op)

            # ---- tail: write h~_T out as fp32 ----
            hout = consts.tile([64, BL], f32, tag="hout", name="hout")
            fin = T % 2
            nc.scalar.copy(hout[0:64, :], hxm[fin][0:64, :])
            nc.gpsimd.dma_start(out=ht_d[:], in_=hout[:])

    nc.compile()
    return nc


def _prep_consts(W_ih, W_hh, b_ih, b_hh):
    bsum = (b_ih + b_hh).astype(np.float64)
    Whh = W_hh.astype(np.float64)
    Wih = W_ih.astype(np.float64)
    # torch gate blocks: i=0:64, f=64:128, g=128:192, o=192:256
    i_s, f_s, g_s, o_s = slice(0, 64), slice(64, 128), slice(128, 192), slice(192, 256)

    def blocks(rows, cscale):
        # h-part [65, 64]: W_hh^T x0.5 (h~ convention) + bias row
        wh = (Whh[rows] * cscale * 0.5).T          # [64, 64]
        bb = (bsum[rows] * cscale)[None, :]        # [1, 64]
        wx = (Wih[rows] * cscale).T                # [8, 64]
        return np.concatenate([wh, bb], 0), wx

    f_h, f_x = blocks(f_s, 0.5)
    i_h, i_x = blocks(i_s, 0.5)
    o_h, o_x = blocks(o_s, 0.5)
    g_h, g_x = blocks(g_s, 1.0)

    w0h = np.concatenate([f_h, i_h], 1)            # [65, 128]  P0 = [f; i]
    w1h = np.concatenate([o_h, g_h], 1)            # [65, 128]  P1 = [o; g]
    w0x_blk = np.concatenate([f_x, i_x], 1)        # [8, 128]
    w1x_blk = np.concatenate([o_x, g_x], 1)        # [8, 128]
    # wxk[k]: block-diagonal selector for sub-step k of an 8-step x group
    wxk = np.zeros((16, 64, 128), np.float64)
    for k in range(8):
        wxk[k, 8 * k:8 * k + 8, :] = w0x_blk
        wxk[8 + k, 8 * k:8 * k + 8, :] = w1x_blk

    aadd = np.zeros((128, 64), np.float64)
    aadd[np.arange(64), np.arange(64)] = 0.5
    aadd[np.arange(64, 128), np.arange(64)] = 0.5
    bf = ml_dtypes.bfloat16
    ident = np.eye(128, dtype=np.float32)
    return tuple(a.astype(bf) for a in (w0h, w1h, wxk, aadd)) + (ident,)


def kernel(x, W_ih, W_hh, b_ih, b_hh, W_fc, b_fc):
    from concourse.bass_utils import run_bass_kernel_spmd

    if _cache["nc"] is None:
        _cache["nc"] = _build_nc()
    nc = _cache["nc"]

    w0h, w1h, wxk, aadd, ident = _prep_consts(
        np.asarray(W_ih, np.float32), np.asarray(W_hh, np.float32),
        np.asarray(b_ih, np.float32), np.asarray(b_hh, np.float32))

    xr = np.asarray(x, np.float32).reshape(B, TI)
    in_maps = []
    for c in range(N_CORES):
        in_maps.append({
            "x": xr[c * BL:(c + 1) * BL],
            "w0h": w0h, "w1h": w1h, "wxk": wxk, "aadd": aadd, "ident": ident,
        })
    res = run_bass_kernel_spmd(nc, in_maps, list(range(N_CORES))).results

    # hT per core: [64, BL] fp32 of h~ = 2h  ->  h [BL, 64]
    h = np.concatenate([0.5 * res[c]["hT"].T for c in range(N_CORES)], 0)
    out = h.astype(np.float32) @ np.asarray(W_fc, np.float32).T + np.asarray(
        b_fc, np.float32)
    return out.astype(np.float32)
